# revision 1
# baseline (speedup 1.0000x reference)
"""DynamicSegmentationHead Trainium2 kernel.

Data-parallel over the 16 clip-frames: each of the 8 NeuronCores handles 2
frames (100 queries). Per core, the dynamic-conv head is evaluated as a chain
of PE matmuls over "query groups":

  controller:  params = hs @ W_ctrl.T + b_ctrl  (host-permuted weight layout
               so params land in device-friendly layouts)
  L1:  y1 = relu(W0eff @ [feat; -gx; -gy; 1])   K=11 (21 for the mixed group)
  L2:  y2 = relu(blockdiag(w1) @ [y1; 1])       K=121
  L3:  out = blockdiag(w2) @ [y2; 1]            K=121, M=15, packed into
       32-aligned PSUM strips, staged to SBUF, DMA'd to DRAM.

Queries are processed in 7 groups of <=15 (6 pure single-frame groups + 1
mixed tail group), pixels in chunks of 1024 (8 full + 448 tail).
"""

import numpy as np

import concourse.bass as bass
import concourse.bacc as bacc
import concourse.tile as tile
from concourse import mybir
from concourse import bass_utils

F32 = mybir.dt.float32
F32R = mybir.dt.float32r

HID = 256
NP = 169
Q = 50
H, W = 72, 120
P = H * W            # 8640
NQ = 100             # queries per core (2 frames)
NCORES = 8
STRIDE = 4

CHUNK = 512
CHUNKS = [(i * CHUNK, CHUNK) for i in range(16)] + [(16 * CHUNK, P - 16 * CHUNK)]
MMN = 512            # matmul free-dim window

# group table: (kind, frame, qbase, nq) ; g6 is the mixed tail group
GROUPS = [(0, 0, 0, 15), (0, 0, 15, 15), (0, 0, 30, 15),
          (1, 1, 0, 15), (1, 1, 15, 15), (1, 1, 30, 15),
          (2, None, 45, 10)]

ACT_OVH, ACT_RATE = 172.0 / 1.2, 1.0 / 1.2     # ns per op / per row (PSUM src)
DVE_OVH, DVE_RATE = 120.0 / 1.2, 1.0 / 0.96


def _windows(clen):
    out = []
    off = 0
    while off < clen:
        wl = min(MMN, clen - off)
        out.append((off, wl))
        off += wl
    return out


def _build_program():
    nc = bacc.Bacc("TRN2", target_bir_lowering=False, debug=False)
    R = lambda ap: ap.bitcast(F32R)

    mf = nc.dram_tensor("mf", [2, 8, P], F32, kind="ExternalInput").ap()
    hsz = nc.dram_tensor("hsz", [HID + 1, NQ], F32, kind="ExternalInput").ap()
    wdev = nc.dram_tensor("wdev", [HID + 1, NP], F32, kind="ExternalInput").ap()
    refs = nc.dram_tensor("refs", [2, 1024], F32, kind="ExternalInput").ap()
    cst = nc.dram_tensor("cst", [4, P], F32, kind="ExternalInput").ap()
    outp = nc.dram_tensor("outp", [NQ, P], F32, kind="ExternalOutput").ap()

    Relu = mybir.ActivationFunctionType.Relu
    Copy = mybir.ActivationFunctionType.Copy

    # greedy ACT/DVE load balancer
    eng_t = {"act": 0.0, "dve": 8000.0}  # DVE pre-loaded: ctrl copies + memsets

    def pick_engine(fd):
        ca = eng_t["act"] + ACT_OVH + fd * ACT_RATE
        cd = eng_t["dve"] + DVE_OVH + fd * DVE_RATE
        if ca <= cd:
            eng_t["act"] = ca
            return "act"
        eng_t["dve"] = cd
        return "dve"

    def relu_to(out_ap, in_ap, fd):
        if pick_engine(fd) == "act":
            nc.scalar.activation(out_ap, in_ap, Relu)
        else:
            nc.vector.tensor_scalar(
                out=out_ap, in0=in_ap, scalar1=0.0, scalar2=None,
                op0=mybir.AluOpType.max)

    def copy_to(out_ap, in_ap, fd):
        if pick_engine(fd) == "act":
            nc.scalar.activation(out_ap, in_ap, Copy)
        else:
            nc.vector.tensor_copy(out_ap, in_ap)

    with tile.TileContext(nc) as tc:
        with tc.tile_pool(name="persist", bufs=1) as pers, \
             tc.tile_pool(name="stg", bufs=4) as stg:

            # ---------------- persistent SBUF ----------------
            Ft = pers.tile([128, P], F32R, tag="F")
            wsb1 = pers.tile([128, NP], F32, tag="wsb1")
            wsb2 = pers.tile([128, NP], F32, tag="wsb2")
            wsb3 = pers.tile([1, NP], F32, tag="wsb3")
            hsb1 = pers.tile([128, NQ], F32, tag="hsb1")
            hsb2 = pers.tile([128, NQ], F32, tag="hsb2")
            hsb3 = pers.tile([1, NQ], F32, tag="hsb3")
            refsb = pers.tile([2, 1024], F32, tag="refsb")
            PW0S = pers.tile([11, 1024], F32, tag="PW0S")
            PW1S = pers.tile([9, 1024], F32, tag="PW1S")
            PW23S = pers.tile([9, NQ], F32, tag="PW23S")
            ctile = pers.tile([1, 1024], F32, tag="ctile")
            vtmp = pers.tile([3, 1024], F32, tag="vtmp")
            w89 = pers.tile([2, 1024], F32, tag="w89")
            b1raw = pers.tile([1, 1024], F32, tag="b1raw")
            ones3 = pers.tile([3, 1], F32, tag="ones3")
            # interleaved staging (all partition-base 0)
            Sw = pers.tile([10, NQ * 8], F32, tag="Sw")     # w0 rows, (n,o)
            Sc = pers.tile([1, NQ * 8], F32, tag="Sc")      # c row, (n,o)
            W1C = pers.tile([8, NQ * 8], F32, tag="W1C")    # w1, (n,o2)
            Sb1 = pers.tile([1, NQ * 8], F32, tag="Sb1")    # b1 row, (n,o2)

            T1 = pers.tile([128, 840], F32R, tag="T1")
            T2 = pers.tile([121, 840], F32R, tag="T2")
            T3 = pers.tile([121, 1120], F32R, tag="T3")
            W1C2 = pers.tile([8, 840], F32, tag="W1C2")   # w1 group-ordered
            W23G = pers.tile([9, 105], F32, tag="W23G")   # w2+b2 group-ordered
            y1l = [pers.tile([121, MMN], F32R, name=f"y1_{i}", tag=f"y1_{i}") for i in range(3)]
            y2l = [pers.tile([121, MMN], F32R, name=f"y2_{i}", tag=f"y2_{i}") for i in range(6)]
            stageA = pers.tile([128, P], F32, tag="stageA")
            stageB = pers.tile([128, P], F32, tag="stageB")

            # ---------------- input DMAs ----------------
            nc.sync.dma_start(out=wsb1[:, :], in_=wdev[0:128, :])
            nc.sync.dma_start(out=wsb2[:, :], in_=wdev[128:256, :])
            nc.sync.dma_start(out=wsb3[:, :], in_=wdev[256:257, :])
            nc.sync.dma_start(out=hsb1[:, :], in_=hsz[0:128, :])
            nc.sync.dma_start(out=hsb2[:, :], in_=hsz[128:256, :])
            nc.sync.dma_start(out=hsb3[:, :], in_=hsz[256:257, :])
            nc.sync.dma_start(out=refsb[:, :], in_=refs)

            # F layout: [0:8]=feat0, [8:10]=-gx,-gy, [10]=ones,
            #           [32:40]=feat1, [40:42]=-gx,-gy, [42]=ones,
            #           [64:72]=feat0, [72:74]=-gx,-gy, [74]=ones,
            #           [75:83]=feat1, [83:85]=-gx,-gy
            nc.sync.dma_start(out=R(Ft[0:8, :]), in_=R(mf[0]))
            nc.sync.dma_start(out=R(Ft[8:10, :]), in_=R(cst[0:2, :]))
            nc.sync.dma_start(out=R(Ft[10:11, :]), in_=R(cst[2:3, :]))
            nc.sync.dma_start(out=R(Ft[32:40, :]), in_=R(mf[1]))
            nc.sync.dma_start(out=R(Ft[40:42, :]), in_=R(cst[0:2, :]))
            nc.sync.dma_start(out=R(Ft[42:43, :]), in_=R(cst[2:3, :]))
            nc.gpsimd.dma_start(out=R(Ft[64:72, :]), in_=R(mf[0]))
            nc.gpsimd.dma_start(out=R(Ft[72:74, :]), in_=R(cst[0:2, :]))
            nc.gpsimd.dma_start(out=R(Ft[74:75, :]), in_=R(cst[2:3, :]))
            nc.gpsimd.dma_start(out=R(Ft[75:83, :]), in_=R(mf[1]))
            nc.gpsimd.dma_start(out=R(Ft[83:85, :]), in_=R(cst[0:2, :]))

            # ---------------- zero fills (POOL memsets; f32 bitcast) -------
            FV = lambda ap: ap.bitcast(F32)
            nc.vector.memset(FV(T1[64:85, 720:800]), 0.0)
            nc.vector.memset(FV(T2[:, :]), 0.0)
            nc.vector.memset(FV(T3[:, :]), 0.0)
            nc.vector.memset(W23G[:, :], 0.0)
            nc.vector.memset(ones3[:, :], 1.0)
            for t in y1l + y2l:
                nc.vector.memset(FV(t[0:120, :]), 0.0)
                nc.gpsimd.dma_start(out=R(t[120:121, :]), in_=R(cst[2:3, 0:MMN]))

            # ---------------- controller matmuls ----------------
            with tc.tile_pool(name="psctrl", bufs=1, space="PSUM") as psc:
                pw0 = psc.tile([11, 1024], F32, tag="pw0")
                pw1 = psc.tile([9, 1024], F32, tag="pw1")
                pw23 = psc.tile([9, NQ], F32, tag="pw23")
                nc.vector.memset(pw0[:, :], 0.0)
                nc.vector.memset(pw1[:, :], 0.0)
                kchunks = [(wsb1, hsb1, 128), (wsb2, hsb2, 128), (wsb3, hsb3, 1)]
                for o in range(8):
                    for kc, (wk, hk, kn) in enumerate(kchunks):
                        nc.tensor.matmul(
                            pw0[0:11, o * 128:o * 128 + NQ],
                            wk[0:kn, o * 11:o * 11 + 11], hk[0:kn, :],
                            start=(kc == 0), stop=(kc == 2))
                for o2 in range(8):
                    for kc, (wk, hk, kn) in enumerate(kchunks):
                        nc.tensor.matmul(
                            pw1[0:9, o2 * 128:o2 * 128 + NQ],
                            wk[0:kn, 88 + o2 * 9:88 + o2 * 9 + 9], hk[0:kn, :],
                            start=(kc == 0), stop=(kc == 2))
                for kc, (wk, hk, kn) in enumerate(kchunks):
                    nc.tensor.matmul(
                        pw23[0:9, 0:NQ],
                        wk[0:kn, 160:169], hk[0:kn, :],
                        start=(kc == 0), stop=(kc == 2))

                nc.vector.tensor_copy(PW0S[:, :], pw0[:, :])
                nc.vector.tensor_copy(PW1S[:, :], pw1[:, :])
                nc.vector.tensor_copy(PW23S[:, :], pw23[:, :])

                # c row: c[o*128+n] = w8*refx + w9*refy + b0
                # PW0S rows: 0:8 = w0 i0..7, 8 = w8, 9 = w9, 10 = b0
                nc.sync.dma_start(out=w89[0:2, :], in_=PW0S[8:10, :])
                nc.vector.tensor_tensor(out=vtmp[0:2, :], in0=w89[0:2, :],
                                        in1=refsb[0:2, :],
                                        op=mybir.AluOpType.mult)
                nc.sync.dma_start(out=vtmp[2:3, :], in_=PW0S[10:11, :])
                pc = psc.tile([1, 1024], F32, tag="pc")
                nc.tensor.matmul(pc[0:1, 0:512], ones3[0:3, 0:1],
                                 vtmp[0:3, 0:512], start=True, stop=True)
                nc.tensor.matmul(pc[0:1, 512:1024], ones3[0:3, 0:1],
                                 vtmp[0:3, 512:1024], start=True, stop=True)
                nc.vector.tensor_copy(ctile[0:1, :], pc[0:1, :])

            # ------------- interleaved staging (DVE, base 0) -------------
            # Sw[r, n*8+o]   = PW0S[r, o*128+n]
            # Sc[0, n*8+o]   = ctile[0, o*128+n]
            # W1C[o, n*8+o2] = PW1S[o, o2*128+n]
            # Sb1[0, n*8+o2] = PW1S[8, o2*128+n]
            w0r = PW0S.rearrange("p (o n) -> p n o", o=8)    # [11, 128, 8]
            cr = ctile.rearrange("p (o n) -> p n o", o=8)    # [1, 128, 8]
            w1r = PW1S.rearrange("p (o n) -> p n o", o=8)    # [9, 128, 8]
            nc.vector.tensor_copy(Sw[0:10, :], w0r[0:10, 0:NQ, :])
            nc.vector.tensor_copy(Sc[0:1, :], cr[0:1, 0:NQ, :])
            nc.sync.dma_start(out=b1raw[0:1, :], in_=PW1S[8:9, :])
            b1r = b1raw.rearrange("p (o n) -> p n o", o=8)
            nc.vector.tensor_copy(Sb1[0:1, :], b1r[0:1, 0:NQ, :])
            # group-ordered stagings: cols (g, j, .) ; g6 at block 6
            for dst0, n0, n1 in ((0, 0, 45), (360, 50, 95),
                                 (720, 45, 50), (760, 95, 100)):
                nc.vector.tensor_copy(W1C2[0:8, dst0:dst0 + (n1 - n0) * 8],
                                      w1r[0:8, n0:n1, :])
            for dst0, n0, n1 in ((0, 0, 45), (45, 50, 95),
                                 (90, 45, 50), (95, 95, 100)):
                nc.vector.tensor_copy(W23G[0:9, dst0:dst0 + n1 - n0],
                                      PW23S[0:9, n0:n1])

            # ------------- per-group weight builds (block DMAs) ----------
            # spread across both HWDGE queues (SP, ACT) and POOL SWDGE
            _brot = [[nc.sync, nc.scalar], [nc.gpsimd]]
            _bi = [0]
            _bphase = [0]

            def bdma(**kw):
                rot = _brot[_bphase[0]]
                e = rot[_bi[0] % len(rot)]
                _bi[0] += 1
                e.dma_start(**kw)

            # Phase-A-critical builds first (T2 blocks, T1 f0/f1, T3-A),
            # then phase-B / mixed builds (hide under phase-A compute).
            bdma(out=R(T1[0:10, 0:360]), in_=R(Sw[0:10, 0:360]))
            bdma(out=R(T1[10:11, 0:360]), in_=R(Sc[0:1, 0:360]))
            bdma(out=R(T1[32:42, 360:720]), in_=R(Sw[0:10, 400:760]))
            bdma(out=R(T1[42:43, 360:720]), in_=R(Sc[0:1, 400:760]))
            # L2 block-diagonal: one DMA per j covering all groups
            for j in range(15):
                gcnt = 7 if j < 10 else 6
                src = bass.AP(tensor=W1C2.tensor, offset=W1C2.offset + j * 8,
                              ap=[[W1C2.ap[0][0], 8], [120, gcnt], [1, 8]])
                dst = bass.AP(tensor=T2.tensor,
                              offset=T2.offset + j * 8 * T2.ap[0][0] + j * 8,
                              ap=[[T2.ap[0][0], 8], [120, gcnt], [1, 8]])
                bdma(out=R(dst), in_=R(src))
            bdma(out=R(T2[120:121, 0:360]), in_=R(Sb1[0:1, 0:360]))
            bdma(out=R(T2[120:121, 360:720]), in_=R(Sb1[0:1, 400:760]))
            # L3 columns phase A (uniform col step 192 across groups 0-3)
            for j in range(15):
                srcA = bass.AP(tensor=W23G.tensor, offset=W23G.offset + j,
                               ap=[[W23G.ap[0][0], 8], [15, 4], [1, 1]])
                dstA = bass.AP(tensor=T3.tensor,
                               offset=T3.offset + j * 8 * T3.ap[0][0] + j,
                               ap=[[T3.ap[0][0], 8], [192, 4], [1, 1]])
                bdma(out=R(dstA), in_=R(srcA))
            dstb2A = bass.AP(tensor=T3.tensor, offset=T3.offset + 120 * T3.ap[0][0],
                             ap=[[T3.ap[0][0], 1], [192, 4], [1, 15]])
            bdma(out=R(dstb2A), in_=R(W23G[8:9, 0:60]))
            # ---- phase-B-only builds (POOL SWDGE; hidden under phase A) ----
            _bphase[0] = 1
            bdma(out=R(T1[64:74, 720:760]), in_=R(Sw[0:10, 360:400]))
            bdma(out=R(T1[74:75, 720:760]), in_=R(Sc[0:1, 360:400]))
            bdma(out=R(T1[75:85, 760:800]), in_=R(Sw[0:10, 760:800]))
            bdma(out=R(T1[74:75, 760:800]), in_=R(Sc[0:1, 760:800]))
            bdma(out=R(T2[120:121, 720:760]), in_=R(Sb1[0:1, 360:400]))
            bdma(out=R(T2[120:121, 760:800]), in_=R(Sb1[0:1, 760:800]))
            for j in range(15):
                cntB = 3 if j < 10 else 2
                srcB = bass.AP(tensor=W23G.tensor, offset=W23G.offset + 60 + j,
                               ap=[[W23G.ap[0][0], 8], [15, cntB], [1, 1]])
                dstB = bass.AP(tensor=T3.tensor,
                               offset=T3.offset + j * 8 * T3.ap[0][0] + 640 + j,
                               ap=[[T3.ap[0][0], 8], [192, cntB], [1, 1]])
                bdma(out=R(dstB), in_=R(srcB))
            dstb2B = bass.AP(tensor=T3.tensor,
                             offset=T3.offset + 120 * T3.ap[0][0] + 640,
                             ap=[[T3.ap[0][0], 1], [192, 3], [1, 15]])
            bdma(out=R(dstb2B), in_=R(W23G[8:9, 60:105]))

            # ---------------- main loop: two phases ----------------
            # Phase A runs groups 0-3 (psum strips 0/32/64/96), phase B runs
            # groups 4-6; phase-B weight builds hide under phase-A compute.
            # Output accumulates into SBUF stage tensors; DRAM writes happen
            # as a few large strip-DMAs per quarter.
            psm_cm = tc.tile_pool(name="psmain", bufs=3, space="PSUM")
            psm = psm_cm.__enter__()
            QUARTERS = [(0, 4), (4, 8), (8, 12), (12, 17)]  # chunk-index spans
            PHASES = [
                # (glist-in-M-desc-order, stage, strip-dmas: (srow, erow, orow0, orow1))
                ([3, 2, 1, 0], stageA,
                 [(0, 15, 0), (32, 47, 15), (64, 79, 30), (96, 111, 50)]),
                ([6, 5, 4], stageB,
                 [(0, 15, 65), (32, 47, 80), (64, 69, 45), (69, 74, 95)]),
            ]
            out_eng = [nc.sync, nc.scalar]
            for pi, (glist, stage_t, strips) in enumerate(PHASES):
                for ci, (coff, clen) in enumerate(CHUNKS):
                    for gi, g in enumerate(sorted(glist)):
                        kind, fr, qb, nq = GROUPS[g]
                        it = ci * len(glist) + gi
                        y1 = y1l[it % 3]
                        y2 = y2l[(ci * 4 + (g % 4)) % 6]
                        m = nq * 8
                        if kind == 0:
                            band, k1 = 0, 11
                        elif kind == 1:
                            band, k1 = 32, 11
                        else:
                            band, k1 = 64, 21
                        ps1 = psm.tile([120, MMN], F32, tag="ps1",
                                       name=f"ps1_{pi}_{it}")
                        nc.tensor.matmul(
                            ps1[0:m, 0:clen],
                            T1[band:band + k1,
                               g * 120:g * 120 + m].bitcast(F32R),
                            Ft[band:band + k1, coff:coff + clen].bitcast(F32R),
                            start=True, stop=True)
                        relu_to(R(y1[0:m, 0:clen]), ps1[0:m, 0:clen], clen)
                        ps2 = psm.tile([120, MMN], F32, tag="ps2",
                                       name=f"ps2_{pi}_{it}")
                        nc.tensor.matmul(
                            ps2[0:m, 0:clen],
                            T2[0:121, g * 120:g * 120 + m].bitcast(F32R),
                            y1[0:121, 0:clen].bitcast(F32R),
                            start=True, stop=True)
                        relu_to(R(y2[0:m, 0:clen]), ps2[0:m, 0:clen], clen)

                    # L3: accumulate strips into one offset-0 psum tile
                    # (fp32r matmuls require dst partition offset 0); descend
                    # by M so start=True initializes the full range.
                    mtop = 32 * (glist[0] % 4) + GROUPS[glist[0]][3]
                    ph = psm.tile([128, MMN], F32, tag="ps3", bufs=2,
                                  name=f"ps3_{pi}_{ci}")
                    for gi, g in enumerate(glist):
                        nq = GROUPS[g][3]
                        y2 = y2l[(ci * 4 + (g % 4)) % 6]
                        m3 = 32 * (g % 4) + nq
                        nc.tensor.matmul(
                            ph[0:m3, 0:clen],
                            T3[0:121, g * 160:g * 160 + m3],
                            y2[0:121, 0:clen],
                            start=(gi == 0), stop=(gi == len(glist) - 1),
                            skip_group_check=True)
                    copy_to(stage_t[0:mtop, coff:coff + clen],
                            ph[0:mtop, 0:clen], clen)

                    # quarter boundary: flush this phase's finished columns
                    for qi, (c0i, c1i) in enumerate(QUARTERS):
                        if ci == c1i - 1:
                            p0 = CHUNKS[c0i][0]
                            p1 = coff + clen
                            for si, (srow, erow, orow) in enumerate(strips):
                                nrow = erow - srow
                                out_eng[si % 2].dma_start(
                                    out=outp[orow:orow + nrow, p0:p1],
                                    in_=stage_t[srow:erow, p0:p1])
            psm_cm.__exit__(None, None, None)

    nc.compile()
    return nc


_NC = None


def _get_nc():
    global _NC
    if _NC is None:
        _NC = _build_program()
    return _NC


def _host_pack(hs, mask_features, references, sizes, W_ctrl, b_ctrl):
    hs = np.asarray(hs, np.float32)
    mask_features = np.asarray(mask_features, np.float32)
    references = np.asarray(references, np.float32)
    sizes = np.asarray(sizes, np.float32)
    W_ctrl = np.asarray(W_ctrl, np.float32)
    b_ctrl = np.asarray(b_ctrl, np.float32)

    # pixel grid
    xs = np.arange(W, dtype=np.float32) * STRIDE + STRIDE // 2
    ys = np.arange(H, dtype=np.float32) * STRIDE + STRIDE // 2
    gxf = np.tile(xs, H)
    gyf = np.repeat(ys, W)
    cstm = np.stack([-gxf, -gyf, np.ones(P, np.float32),
                     np.zeros(P, np.float32)]).astype(np.float32)

    # W permutation
    W_aug = np.concatenate([W_ctrl.T, b_ctrl[None, :]], 0)  # [257, 169]
    perm = []
    for o in range(8):
        perm += [o * 10 + i for i in range(10)] + [152 + o]
    for o2 in range(8):
        perm += [80 + o2 * 8 + oo for oo in range(8)] + [160 + o2]
    perm += [144 + oo for oo in range(8)] + [168]
    wdev = np.ascontiguousarray(W_aug[:, perm])

    # reference points in pixels
    b_idx = np.arange(16) // 8
    scale = sizes[b_idx][:, ::-1]                      # [16, 2] = (img_w, img_h)
    refs_px = references * scale[:, None, :]           # [16, 50, 2]

    in_maps = []
    for c in range(NCORES):
        hs_c = hs[2 * c:2 * c + 2].reshape(NQ, HID)
        hsz = np.concatenate([hs_c.T, np.ones((1, NQ), np.float32)], 0)
        mf_c = mask_features[2 * c:2 * c + 2].reshape(2, 8, P)
        rp = refs_px[2 * c:2 * c + 2].reshape(NQ, 2)
        refs_rep = np.zeros((2, 1024), np.float32)
        for o in range(8):
            refs_rep[0, o * 128:o * 128 + NQ] = rp[:, 0]
            refs_rep[1, o * 128:o * 128 + NQ] = rp[:, 1]
        in_maps.append(dict(
            mf=np.ascontiguousarray(mf_c),
            hsz=np.ascontiguousarray(hsz),
            wdev=wdev,
            refs=refs_rep,
            cst=cstm,
        ))
    return in_maps


def kernel(hs, mask_features, references, sizes, W_ctrl, b_ctrl, T):
    assert int(T) == 8
    nc = _get_nc()
    in_maps = _host_pack(hs, mask_features, references, sizes, W_ctrl, b_ctrl)
    res = bass_utils.run_bass_kernel_spmd(nc, in_maps, core_ids=list(range(NCORES)))
    out = np.empty((16, Q, H, W), np.float32)
    for c in range(NCORES):
        out[2 * c:2 * c + 2] = res.results[c]["outp"].reshape(2, Q, H, W)
    return out

